# revision 17
# baseline (speedup 1.0000x reference)
"""GRNNCell (group-equivariant RNN cell) Trainium2 kernel.

out[b,g,e] = tanh( sum_{g',i} input[b,g',i]  W_w[(g-g')%G][i,e]
                 + sum_{g',i} hidden[b,g',i] W_h[(g-g')%G][i,e]
                 + b_w[e] + b_h[e] )

Strategy
--------
* Concatenate (input, hidden) along i and (W_w, W_h) along their input dim:
  one block-circulant GEMM with K = G*2H = 12288 per output g.
* Shard 8 cores = 4 batch blocks (128 rows) x 2 output-e halves (256 cols).
* Host stages transposed/bf16 layouts so the device never transposes:
    xt[isub] : (128, G*128)  X^T chunk  [i_local, (g', b)]
    wt[isub] : (128, 13*256) W_cat chunk [i_local, (j, e)], j=12 dup of j=0
  Matmuls: psum[pair] += xt_chunk.T @ wt[j:j+2]  (M=128 b, N=512 = two g's).
  The g-pair trick keeps every matmul at N=512 (full PSUM bank).
* Bias via K=1 matmul; ones row and bias row are baked into the tails of the
  xt/wt half tensors (keeps DMA count at 5 so no DMA ever needs 2 sem waits:
  walrus's DIRECT2D DMA lowering supports only one, and the final drain ~8).
* tanh fused on ScalarE from PSUM; single output DMA.
"""

import numpy as np
import ml_dtypes

G = 12
H = 512
B = 512
P = 4            # batch shards
Q = 2            # e shards
BL = B // P      # 128 rows per core
EL = H // Q      # 256 out-cols per core
KI = 2 * H       # 1024 contraction (input+hidden)
NSUB = KI // 128 # 8 k-chunks
NPAIR = G // 2   # 6 g-pairs
HS = NSUB // 2   # 4 k-chunks per DMA half

XH = HS * G * BL          # xt half cols (no tail)
WH = HS * 13 * EL         # wt half cols (no tail)

COMPUTE_DT = "float32r"   # "bfloat16" | "float32r" | "float32"

_CACHE = {}


def _np_dt(name):
    return ml_dtypes.bfloat16 if name == "bfloat16" else np.float32


def _build(dt_name):
    import concourse.bass as bass
    import concourse.mybir as mybir

    cdt = getattr(mybir.dt, dt_name)
    f32 = mybir.dt.float32
    C0 = XH + WH + BL + 2 * EL
    C1 = XH + WH
    nc = bass.Bass()
    # chunk0 = [xt half | wt half | ones row | bias row], chunk1 = [xt | wt]
    ab0_d = nc.declare_dram_parameter("ab0", [128, C0], cdt, isOutput=False)
    ab1_d = nc.declare_dram_parameter("ab1", [128, C1], cdt, isOutput=False)
    out_d = nc.declare_dram_parameter("out", [128, G * EL], f32, isOutput=True)

    with (
        nc.semaphore("dsem0") as dsem0,
        nc.semaphore("dsem1") as dsem1,
        nc.semaphore("psem") as psem,
        nc.semaphore("asem") as asem,
        nc.semaphore("osem") as osem,
        nc.sbuf_tensor("ab0_sb", [128, C0], cdt) as ab0,
        nc.sbuf_tensor("ab1_sb", [128, C1], cdt) as ab1,
        nc.psum_tensor("acc", [128, NPAIR * 2 * EL], f32) as acc,
        nc.sbuf_tensor("osb", [128, G * EL], f32) as osb,
    ):
        with nc.Block() as block:

            @block.sync
            def _(sync):
                sync.dma_start(out=ab0[:], in_=ab0_d[:]).then_inc(dsem0, 16)
                sync.dma_start(out=ab1[:], in_=ab1_d[:]).then_inc(dsem1, 16)
                sync.wait_ge(asem, 1)
                sync.dma_start(out=out_d[:], in_=osb[:]).then_inc(osem, 16)
                sync.wait_ge(osem, 16)

            @block.tensor
            def _(pe):
                for isub in range(NSUB):
                    ab = ab0 if isub < HS else ab1
                    if isub == 0:
                        pe.wait_ge(dsem0, 16)
                    if isub == HS:
                        pe.wait_ge(dsem1, 16)
                    il = isub % HS
                    for gp in range(G):
                        lhsT = ab[:, (il * G + gp) * BL : (il * G + gp + 1) * BL]
                        for t in range(NPAIR):
                            j = (2 * t - gp) % G
                            rhs = ab[
                                :,
                                XH + (il * 13 + j) * EL : XH + (il * 13 + j + 2) * EL,
                            ]
                            pe.matmul(
                                acc[:, t * 2 * EL : (t + 1) * 2 * EL],
                                lhsT,
                                rhs,
                                start=(isub == 0 and gp == 0),
                                stop=False,
                            )
                ones_row = ab0[0:1, XH + WH : XH + WH + BL]
                bias_row = ab0[0:1, XH + WH + BL : XH + WH + BL + 2 * EL]
                for t in range(NPAIR):
                    mm = pe.matmul(
                        acc[:, t * 2 * EL : (t + 1) * 2 * EL],
                        ones_row,
                        bias_row,
                        start=False,
                        stop=True,
                    )
                mm.then_inc(psem, 1)

            @block.scalar
            def _(act):
                act.wait_ge(psem, 1)
                act.activation(
                    osb[:], acc[:], mybir.ActivationFunctionType.Tanh
                ).then_inc(asem, 1)

    return nc


def _stage_inputs(input, hidden, W_w, b_w, W_h, b_h):
    """Host-side layout staging: per-core in_maps (content differs per core)."""
    np_dt = _np_dt(COMPUTE_DT)
    w_cat = np.concatenate([W_w, W_h], axis=1)       # (G, KI, H) [j, i, e]
    bias = (b_w + b_h).astype(np.float64)

    in_maps = []
    for c in range(P * Q):
        pb, qe = divmod(c, Q)
        bsl = slice(pb * BL, (pb + 1) * BL)
        esl = slice(qe * EL, (qe + 1) * EL)

        x_cat = np.concatenate([input[bsl], hidden[bsl]], axis=2)  # (BL, G, KI)
        # -> [i, g', b] -> (NSUB, 128, G*BL)
        xt = np.ascontiguousarray(x_cat.transpose(2, 1, 0)).reshape(NSUB, 128, G * BL)
        xa = np.ascontiguousarray(xt[:HS].transpose(1, 0, 2)).reshape(128, XH)
        xb = np.ascontiguousarray(xt[HS:].transpose(1, 0, 2)).reshape(128, XH)

        w_loc = w_cat[:, :, esl]                                   # (G, KI, EL)
        w13 = np.concatenate([w_loc, w_loc[:1]], axis=0)           # (13, KI, EL)
        wt = np.ascontiguousarray(w13.transpose(1, 0, 2)).reshape(NSUB, 128, 13 * EL)
        wa = np.ascontiguousarray(wt[:HS].transpose(1, 0, 2)).reshape(128, WH)
        wb = np.ascontiguousarray(wt[HS:].transpose(1, 0, 2)).reshape(128, WH)

        ones = np.ones((128, BL), np.float64)
        brow = np.tile(np.tile(bias[esl], 2).reshape(1, 2 * EL), (128, 1))
        ab0 = np.concatenate([xa, wa, ones, brow], axis=1)
        ab1 = np.concatenate([xb, wb], axis=1)

        in_maps.append(
            {
                "ab0": ab0.astype(np_dt),
                "ab1": ab1.astype(np_dt),
            }
        )
    return in_maps


def _run(in_maps, trace=False):
    from concourse.bass_utils import run_bass_kernel_spmd

    key = COMPUTE_DT
    if key not in _CACHE:
        _CACHE[key] = _build(COMPUTE_DT)
    nc = _CACHE[key]
    return run_bass_kernel_spmd(nc, in_maps, list(range(P * Q)), trace=trace)


def kernel(input, hidden, W_w, b_w, W_h, b_h, perm, _trace=False):
    in_maps = _stage_inputs(
        np.asarray(input, np.float32),
        np.asarray(hidden, np.float32),
        np.asarray(W_w, np.float32),
        np.asarray(b_w, np.float32),
        np.asarray(W_h, np.float32),
        np.asarray(b_h, np.float32),
    )
    res = _run(in_maps, trace=_trace)
    out = np.empty((B, G, H), np.float32)
    for c in range(P * Q):
        pb, qe = divmod(c, Q)
        blk = res.results[c]["out"].reshape(BL, G, EL)
        out[pb * BL : (pb + 1) * BL, :, qe * EL : (qe + 1) * EL] = blk
    if _trace:
        kernel._last = res
    return out
